# revision 32
# baseline (speedup 1.0000x reference)
"""Causal self-attention (B=4, T=2048, C=1024, H=16, D=64) on 8 TRN2 NeuronCores.

Sharding: tensor-parallel over heads - each core owns 2 of the 16 heads.
Per core:
  qkv^T = W_pack.T @ x^T        (x^T streamed, W stationary; q/k/v each [2D, BT])
  S^T   = k_h^T.T @ q_h^T       (per batch, causal blocks only; the two heads
                                 run concurrently as K=64 row-tiles of the PE)
          + TRI.T @ I           (diagonal blocks: -60000 accumulated into the
                                 masked upper triangle while still in PSUM, so
                                 exp() maps it to 0 - no DVE mask op needed)
  P^T   = exp(S^T/sqrt(D))      (no max-subtraction: logits are O(5))
  yu^T  = [v_h | 1].T @ P^T     (ones column accumulates the softmax denom)
  y^T   = yu^T * (1/denom)      (recip via PE row<->col transposes + DVE)
  out_p = y^T.T @ W_proj_rows   (partial over this core's head-rows, fp16)
Host: out = sum over cores of out_p.

The emission is software-pipelined at instruction granularity: the attention
inner loop is ACT(exp)-bound, so qkv matmuls of batch b+1 and proj matmuls
of batch b (one chunk behind) are interleaved into the attention stream of
batch b to fill PE/DVE slack.  A reserve of proj tiles is held back to the
very end so the tail (last chunk normalize chain) overlaps dense PE work and
the HAM clock gate never re-throttles.
"""

import sys

sys.path.insert(0, "/opt/trn_rl_repo")

import numpy as np
import ml_dtypes

import concourse.bass as bass
import concourse.bacc as bacc
import concourse.mybir as mybir
import concourse.tile as tile
from concourse.bass_utils import run_bass_kernel_spmd

BF16 = mybir.dt.bfloat16
F16 = mybir.dt.float16
F32 = mybir.dt.float32
AF = mybir.ActivationFunctionType

N_CORES = 8
N_HEAD = 16
N_EMBD = 1024
HEAD_DIM = N_EMBD // N_HEAD


class Cfg:
    def __init__(self, B=4, T=2048, C=1024, D=64, CH=512, TG=1024):
        self.B, self.T, self.C, self.D, self.CH, self.TG = B, T, C, D, CH, TG
        self.BT = B * T
        self.n_ct = C // 128          # contraction tiles for qkv
        self.nt = T // 128            # 128-row t-tiles per batch
        self.ncw = T // CH            # tq chunks per batch
        self.r = CH // 128            # t-tiles per chunk
        self.ngb = T // TG            # t-groups per batch (qkv phase)
        self.nchp = TG // CH          # chunks per t-group
        assert C % 128 == 0 and T % CH == 0 and CH % 128 == 0 and T % TG == 0
        assert TG % CH == 0 and D == 64
        assert CH // 128 == 4         # nq=4 assumed by the denom gather


def build(cfg: Cfg) -> bacc.Bacc:
    B, T, C, D, CH, TG = cfg.B, cfg.T, cfg.C, cfg.D, cfg.CH, cfg.TG
    BT, n_ct, nt, ncw, r = cfg.BT, cfg.n_ct, cfg.nt, cfg.ncw, cfg.r
    sm_scale = 1.0 / float(np.sqrt(D))
    nq = CH // 128

    nc = bacc.Bacc("TRN2", target_bir_lowering=False, debug=False,
                   num_devices=N_CORES)

    xT_d = nc.dram_tensor("xT", [C, BT], BF16, kind="ExternalInput")
    wq_d = nc.dram_tensor("wq", [128, n_ct * 128], BF16, kind="ExternalInput")
    wk_d = nc.dram_tensor("wk", [128, n_ct * 128], BF16, kind="ExternalInput")
    wv_d = nc.dram_tensor("wv", [128, n_ct * 128], BF16, kind="ExternalInput")
    wp_d = nc.dram_tensor("wp", [128, C], BF16, kind="ExternalInput")
    trm_d = nc.dram_tensor("trm", [128, 128], BF16, kind="ExternalInput")
    idn_d = nc.dram_tensor("idn", [128, 256], BF16, kind="ExternalInput")
    idf_d = nc.dram_tensor("idf", [128, 128], F32, kind="ExternalInput")
    out_d = nc.dram_tensor("outp", [BT, C], F16, kind="ExternalOutput")

    with tile.TileContext(nc) as tc:
        with (
            tc.tile_pool(name="persist", bufs=1) as persist,
            tc.tile_pool(name="xt", bufs=2 * n_ct) as xt_pool,
            tc.tile_pool(name="pp", bufs=6) as p_pool,
            tc.tile_pool(name="vaug", bufs=2) as vaug_pool,
            tc.tile_pool(name="small", bufs=4) as small_pool,
            tc.tile_pool(name="rep", bufs=4) as rep_pool,
            tc.tile_pool(name="tmp1", bufs=4) as tmp_pool,
            tc.tile_pool(name="ob", bufs=6) as ob_pool,
            tc.tile_pool(name="ps_s", bufs=2, space="PSUM") as ps_s,
            tc.tile_pool(name="ps_aux", bufs=2, space="PSUM") as ps_aux,
            tc.tile_pool(name="ps_yu0", bufs=1, space="PSUM") as ps_yu0,
            tc.tile_pool(name="ps_yu1", bufs=1, space="PSUM") as ps_yu1,
        ):
            # ---- persistent SBUF tensors -------------------------------
            qTs = [persist.tile([128, T], BF16, tag=f"qT{b}", name=f"qT{b}")
                   for b in range(B)]
            kTs = [persist.tile([128, T], BF16, tag=f"kT{b}", name=f"kT{b}")
                   for b in range(B)]
            vTs = [persist.tile([128, T], BF16, tag=f"vT{b}", name=f"vT{b}")
                   for b in range(B)]
            yuTs = [persist.tile([128, T], BF16, tag=f"yuT{b}", name=f"yuT{b}")
                    for b in range(B)]
            wq_sb = persist.tile([128, n_ct * 128], BF16, tag="wq")
            wk_sb = persist.tile([128, n_ct * 128], BF16, tag="wk")
            wv_sb = persist.tile([128, n_ct * 128], BF16, tag="wv")
            wp_sb = persist.tile([128, C], BF16, tag="wp")
            trm_sb = persist.tile([128, 128], BF16, tag="trm")
            idn_sb = persist.tile([128, 256], BF16, tag="idn")
            idf_sb = persist.tile([128, 128], F32, tag="idf")
            nc.sync.dma_start(wq_sb[:], wq_d[:])
            nc.sync.dma_start(wk_sb[:], wk_d[:])
            nc.sync.dma_start(wv_sb[:], wv_d[:])
            nc.sync.dma_start(idn_sb[:], idn_d[:])
            nc.sync.dma_start(trm_sb[:], trm_d[:])
            nc.sync.dma_start(idf_sb[:], idf_d[:])
            nc.sync.dma_start(wp_sb[:], wp_d[:])

            # ---- thunk streams -----------------------------------------
            # Each stream is a list of zero-arg emitters; the scheduler
            # interleaves them so each engine's FIFO gets work in an order
            # that keeps all engines fed.

            def qkv_thunks(b):
                """qkv projections for batch b: per t-group, 8 xT DMAs then
                6 units of (8 accumulating matmuls + 1 PSUM->SBUF cast).
                Returns one thunk list per t-group so the scheduler can
                place group 1 half a batch later than group 0."""
                groups = []
                for gl in range(cfg.ngb):
                    thunks = []
                    g0 = b * T + gl * TG
                    l0 = gl * TG
                    xts = []

                    def dma_group(g0=g0, xts=xts, lo=0, hi=n_ct // 2):
                        for ci in range(lo, hi):
                            xt = xt_pool.tile([128, TG], BF16, tag="xt",
                                              name="xt")
                            nc.sync.dma_start(
                                xt[:], xT_d[128 * ci:128 * (ci + 1),
                                            g0:g0 + TG])
                            xts.append(xt)
                    thunks.append(dma_group)
                    thunks.append(lambda g0=g0, xts=xts:
                                  dma_group(g0, xts, n_ct // 2, n_ct))
                    for ch in range(cfg.nchp):
                        for wsb, dsts in ((wq_sb, qTs), (wk_sb, kTs),
                                          (wv_sb, vTs)):
                            box = {}

                            def unit_a(wsb=wsb, ch=ch, xts=xts, box=box):
                                ps = ps_aux.tile([128, CH], F32, tag="aux",
                                                 name="ps")
                                box["ps"] = ps
                                for ci in range(n_ct // 2):
                                    nc.tensor.matmul(
                                        ps[:],
                                        wsb[:, 128 * ci:128 * (ci + 1)],
                                        xts[ci][:, ch * CH:(ch + 1) * CH],
                                        start=(ci == 0), stop=False)

                            def unit_b(wsb=wsb, dsts=dsts, ch=ch, l0=l0,
                                       xts=xts, box=box):
                                ps = box.pop("ps")
                                for ci in range(n_ct // 2, n_ct):
                                    nc.tensor.matmul(
                                        ps[:],
                                        wsb[:, 128 * ci:128 * (ci + 1)],
                                        xts[ci][:, ch * CH:(ch + 1) * CH],
                                        start=False,
                                        stop=(ci == n_ct - 1))
                                nc.vector.tensor_copy(
                                    dsts[b][:, l0 + ch * CH:
                                            l0 + (ch + 1) * CH], ps[:])
                            thunks.append(unit_a)
                            thunks.append(unit_b)
                    groups.append(thunks)
                return groups

            def attn_thunks(b, proj_sink):
                """Attention for batch b. proj_sink(m) is called when t-tile
                m of yuT[b] is final, enabling the proj of that tile.
                Returns (thunks, fin_thunk): fin_thunk is the last chunk's
                normalize chain, to be spliced into the NEXT batch's stream
                (or the tail) so its PE transposes never head-block."""
                qT, kT, vT, yuT = qTs[b], kTs[b], vTs[b], yuTs[b]
                thunks = []
                va = {}

                def prep(va=va):
                    v = vaug_pool.tile([128, nt * 130], BF16, tag="vaug",
                                       name="va")
                    ones = v.rearrange("p (i h c) -> p i h c",
                                       h=2, c=65)[:, :, :, 64]
                    nc.vector.memset(ones, 1.0)
                    va["t"] = v
                thunks.append(prep)

                def vtrans(i, va=va):
                    # one [128,128] PE transpose covers both heads' v
                    vtp = ps_aux.tile([128, 128], BF16, tag="aux",
                                      name="vtp")
                    nc.tensor.transpose(
                        vtp[:], vT[:, 128 * i:128 * (i + 1)],
                        idn_sb[:, 0:128])
                    dst = va["t"].rearrange("p (i h c) -> p i h c",
                                            h=2, c=65)[:, i, :, 0:64]
                    src = vtp.rearrange("k (h d) -> k h d", d=64)
                    nc.vector.tensor_copy(dst, src)

                state = {}

                def s_exp(j, i, state=state):
                    c0 = 128 * (i - r * j) if i >= r * j else 0
                    w = CH - c0
                    diag = i >= r * j
                    ss = ps_s.tile([128, 2 * CH], F32, tag="s", name="ss")
                    for h in (0, 1):
                        nc.tensor.matmul(
                            ss[:, h * CH:h * CH + w],
                            kT[64 * h:64 * h + 64, 128 * i:128 * (i + 1)],
                            qT[64 * h:64 * h + 64,
                               CH * j + c0:CH * (j + 1)],
                            start=True, stop=not diag)
                    if diag:
                        # accumulate -60000 into the masked (strictly
                        # future) triangle of both heads' diagonal blocks
                        # in ONE matmul (rhs = [I|I]); stays on the PE
                        dv = ss.rearrange("p (h c) -> p h c",
                                          c=CH)[:, :, 0:128]
                        nc.tensor.matmul(dv, trm_sb[:], idn_sb[:, 0:256],
                                         start=False, stop=True)
                    pt_ = p_pool.tile([128, 2 * w], BF16, tag="p",
                                      name="pt_")
                    if w == CH:
                        nc.scalar.activation(pt_[:], ss[:], AF.Exp,
                                             scale=sm_scale)
                    else:
                        sv = ss.rearrange("p (h c) -> p h c",
                                          c=CH)[:, :, 0:w]
                        pv = pt_.rearrange("p (h c) -> p h c", c=w)
                        nc.scalar.activation(pv, sv, AF.Exp, scale=sm_scale)
                    state[(j, i)] = pt_

                def pv1(j, i, h, i_max, va=va, state=state):
                    c0 = 128 * (i - r * j) if i >= r * j else 0
                    w = CH - c0
                    pt_ = state[(j, i)]
                    if h == 1:
                        state.pop((j, i))
                    yub = state[("yu", j)]
                    nc.tensor.matmul(
                        yub[h][:, c0:CH],
                        va["t"][:, 130 * i + 65 * h:
                                130 * i + 65 * h + 65],
                        pt_[:, h * w:(h + 1) * w],
                        start=(i == 0), stop=(i == i_max))

                def final_a_last(j, state=state):
                    # last chunk of the last batch: stage UNNORMALIZED yu
                    # and just the reciprocal columns; normalization folds
                    # into the per-head proj matmuls (no broadcast chain)
                    yub = state.pop(("yu", j))
                    yus = []
                    for h in (0, 1):
                        yc = small_pool.tile([65, CH], F32, tag=f"yus{h}",
                                             name=f"yus{h}")
                        nc.vector.tensor_copy(yc[:], yub[h][:])
                        yus.append(yc)
                    cols = slice(CH * j, CH * (j + 1))
                    nc.vector.tensor_copy(yuT[0:64, cols], yus[0][0:64, :])
                    tm = tmp_pool.tile([64, CH], BF16, tag="tmp1")
                    nc.vector.tensor_copy(tm[:], yus[1][0:64, :])
                    nc.sync.dma_start(yuT[64:128, cols], tm[:])
                    state[("lfin", j)] = yus

                def final_b_last(j, state=state):
                    yus = state.pop(("lfin", j))
                    dt = ps_aux.tile([128, 8], F32, tag="aux", name="dt")
                    for h in (0, 1):
                        for k in range(nq):
                            nc.tensor.transpose(
                                dt[:, nq * h + k:nq * h + k + 1],
                                yus[h][64:65, 128 * k:128 * (k + 1)],
                                idf_sb[64:65, 64:65])
                    rcl = small_pool.tile([128, 8], F32, tag="rcolL",
                                          name="rcl")
                    nc.vector.reciprocal(rcl[:], dt[:])
                    last_rcol[0] = rcl

                def final_a(j, state=state):
                    yub = state.pop(("yu", j))
                    # stage yu (+denom row) out of PSUM, then gather both
                    # denom rows into one [8,128] tile for a single transpose
                    yus = []
                    for h in (0, 1):
                        yc = small_pool.tile([65, CH], F32, tag=f"yus{h}",
                                             name=f"yus{h}")
                        nc.vector.tensor_copy(yc[:], yub[h][:])
                        yus.append(yc)
                    dcol = small_pool.tile([8, 128], F32, tag="dcol",
                                           name="dcol")
                    for h in (0, 1):
                        nc.sync.dma_start(dcol[4 * h:4 * h + 4, :],
                                          yus[h][64:65, :])
                    state[("fin", j)] = (dcol, yus)

                def final_b(j, state=state):
                    dcol, yus = state.pop(("fin", j))
                    dt = ps_aux.tile([128, 8], F32, tag="aux", name="dt")
                    nc.tensor.transpose(dt[:], dcol[:], idf_sb[0:8, 0:8])
                    rcol = small_pool.tile([128, 8], F32, tag="rcol")
                    nc.vector.reciprocal(rcol[:], dt[:])
                    rb = ps_aux.tile([8, 128], F32, tag="aux", name="rb")
                    nc.tensor.transpose(rb[:], rcol[:], idf_sb[:, :])
                    rbs = small_pool.tile([8, 128], F32, tag="rbs")
                    nc.vector.tensor_copy(rbs[:], rb[:])
                    rec2 = small_pool.tile([1, 2 * CH], F32, tag="rec2",
                                           name="rec2")
                    nc.sync.dma_start(rec2[0:1, :], rbs[:, :])
                    rep2 = rep_pool.tile([64, 2 * CH], F32, tag="rep",
                                         name="rep2")
                    nc.gpsimd.partition_broadcast(rep2[:], rec2[0:1, :])
                    cols = slice(CH * j, CH * (j + 1))
                    nc.vector.tensor_mul(
                        yuT[0:64, cols], yus[0][0:64, :], rep2[:, 0:CH])
                    tm = tmp_pool.tile([64, CH], BF16, tag="tmp1")
                    nc.vector.tensor_mul(tm[:], yus[1][0:64, :],
                                         rep2[:, CH:2 * CH])
                    nc.sync.dma_start(yuT[64:128, cols], tm[:])

                # stitch the per-chunk streams with PV lagging one i-tile;
                # the denominator/normalize chain of chunk j overlaps the
                # first steps of chunk j+1 so the PE FIFO never head-blocks
                def enable_proj(j):
                    for m in range(nq * j, nq * (j + 1)):
                        proj_sink(m)

                marks = []
                for j in range(ncw):
                    i_max = r * (j + 1) - 1
                    marks.append(len(thunks))

                    def chunk_start(j=j, state=state):
                        state[("yu", j)] = [
                            ps_yu0.tile([65, CH], F32, tag="yu0",
                                        name="yu0"),
                            ps_yu1.tile([65, CH], F32, tag="yu1",
                                        name="yu1")]
                    thunks.append(chunk_start)
                    for k in range(4):
                        thunks.append(lambda i=r * j + k: vtrans(i))
                    for i in range(0, i_max + 1, 2):
                        def step_a(j=j, i=i):
                            s_exp(j, i)
                        def step_b(j=j, i=i):
                            s_exp(j, i + 1)

                        def pv4(j=j, i=i, i_max=i_max):
                            # the previous pair's PVs, grouped per head so
                            # consecutive matmuls hit the SAME PSUM bank
                            # (bank switches cost ~70ns each)
                            for h in (0, 1):
                                for ii in (i - 2, i - 1):
                                    if ii >= 0:
                                        pv1(j, ii, h, i_max)
                        step_a.weight = 0.7
                        step_b.weight = 0.7
                        pv4.weight = 1.2 + 1.5 * (max(0, i - r * j) / r)
                        thunks.append(step_a)
                        thunks.append(step_b)
                        thunks.append(pv4)
                        if j > 0 and i == i_max - 1:
                            # the previous chunk's normalize runs here, a
                            # full chunk after its dcol DMA was dispatched,
                            # so the PE transposes inside never head-block
                            def fin_prev(j=j):
                                final_b(j - 1)
                                enable_proj(j - 1)
                            thunks.append(fin_prev)

                    def tail_pv(j=j, i_max=i_max):
                        for h in (0, 1):
                            for ii in (i_max - 1, i_max):
                                pv1(j, ii, h, i_max)
                    thunks.append(tail_pv)
                    if b == B - 1 and j == ncw - 1:
                        def tail_l(j=j):
                            final_a_last(j)
                        thunks.append(tail_l)
                    else:
                        def tail(j=j):
                            final_a(j)
                        thunks.append(tail)

                if b == B - 1:
                    def fin_last(j=ncw - 1):
                        final_b_last(j)
                        enable_proj(j)
                else:
                    def fin_last(j=ncw - 1):
                        final_b(j)
                        enable_proj(j)
                return thunks, fin_last, marks

            # global proj work queue: halves append as their yuT chunk
            # normalizes; the scheduler drains a couple per attention thunk
            proj_pending = []
            last_rcol = [None]

            proj_obs = {}

            def proj_emit_half(b, m, ch, pool=None, sc=None,
                               dma_half=False):
                if ch == 0:
                    ob = ob_pool.tile([128, C], F16, tag="ob", name="ob")
                    proj_obs[(b, m)] = ob
                else:
                    ob = proj_obs.pop((b, m))
                if b == B - 1 and m >= nq * (ncw - 1):
                    # tail path: yuT holds unnormalized yu; run the two
                    # heads as concurrent K=64 row-tiles and normalize with
                    # per-partition reciprocal columns
                    k = m - nq * (ncw - 1)
                    rcl = last_rcol[0]
                    po0 = ps_aux.tile([128, CH], F32, tag="aux", name="po0")
                    po1 = ps_s.tile([128, 2 * CH], F32, tag="s", name="ss")
                    po1 = po1.rearrange("p (t c) -> p t c", t=2)[:, 0, :]
                    for h, po_ in ((0, po0[:]), (1, po1)):
                        nc.tensor.matmul(
                            po_,
                            yuTs[b][64 * h:64 * h + 64,
                                    128 * m:128 * (m + 1)],
                            wp_sb[64 * h:64 * h + 64,
                                  ch * CH:(ch + 1) * CH],
                            start=True, stop=True)
                    t0 = tmp_pool.tile([128, CH], F32, tag="t0", name="t0")
                    nc.scalar.activation(t0[:], po0[:], AF.Copy,
                                         scale=rcl[:, k:k + 1])
                    nc.vector.scalar_tensor_tensor(
                        ob[:, ch * CH:(ch + 1) * CH], po1,
                        rcl[:, nq + k:nq + k + 1], t0[:],
                        op0=mybir.AluOpType.mult, op1=mybir.AluOpType.add)
                else:
                    if pool is ps_s:
                        # tail only: borrow the (idle) attention S pool,
                        # using its existing tag/shape so it does not grow
                        po = ps_s.tile([128, 2 * CH], F32, tag="s",
                                       name="ss")
                        po = po.rearrange("p (t c) -> p t c", t=2)[:, 0, :]
                    else:
                        po = ps_aux.tile([128, CH], F32, tag="aux",
                                         name="po")[:]
                    nc.tensor.matmul(
                        po,
                        yuTs[b][:, 128 * m:128 * (m + 1)],
                        wp_sb[:, ch * CH:(ch + 1) * CH],
                        start=True, stop=True)
                    use_sc = sc if sc is not None else (2 * m + ch) % 4 == 3
                    if use_sc:
                        nc.scalar.copy(ob[:, ch * CH:(ch + 1) * CH], po)
                    else:
                        nc.vector.tensor_copy(ob[:, ch * CH:(ch + 1) * CH],
                                              po)
                if dma_half:
                    # tail: per-half DMA starts the transfer ~0.7us sooner,
                    # shrinking the end-of-kernel queue flush
                    nc.sync.dma_start(
                        out_d[b * T + 128 * m:b * T + 128 * (m + 1),
                              ch * CH:(ch + 1) * CH],
                        ob[:, ch * CH:(ch + 1) * CH])
                elif ch == 1:
                    nc.sync.dma_start(
                        out_d[b * T + 128 * m:b * T + 128 * (m + 1), :],
                        ob[:])

            drain_clock = [0]

            def proj_sink_for(b):
                def sink(m):
                    # halves become drain-eligible only ~8 attention thunks
                    # after their normalize chain was issued, so they never
                    # head-block the PE FIFO while the chain is in flight
                    proj_pending.append((drain_clock[0] + 8, b, m, 0))
                    proj_pending.append((drain_clock[0] + 8, b, m, 1))
                return sink

            # ---- interleaved scheduler ---------------------------------
            # warmup: ~7us of throwaway matmuls on a scratch tile (no DMA
            # dependency, so HAM unthrottles while weights are in flight)
            wsc = tmp_pool.tile([128, CH], BF16, tag="wsc", name="wsc")
            nc.vector.memset(wsc[:], 0.5)
            warm = ps_aux.tile([128, CH], F32, tag="aux", name="warm")
            for _ in range(24):
                nc.tensor.matmul(warm[:], wsc[:, 0:128], wsc[:],
                                 start=True, stop=True)

            attn_streams = []
            fin_lasts = []
            chunk_marks = []
            for b in range(B):
                th, fl, marks = attn_thunks(b, proj_sink_for(b))
                attn_streams.append(th)
                fin_lasts.append(fl)
                chunk_marks.append(marks)
            qkv_groups = [qkv_thunks(b) for b in range(B)]
            proj_cursor = [0]

            RESERVE = 20   # proj halves held back for the tail

            def drain_proj(limit, force=False, reserve=0, pool=None,
                           sc=None):
                n = 0
                while proj_cursor[0] < len(proj_pending) - reserve \
                        and n < limit:
                    at, b, m, ch = proj_pending[proj_cursor[0]]
                    if not force and at > drain_clock[0]:
                        break
                    proj_emit_half(b, m, ch, pool=pool, sc=sc)
                    proj_cursor[0] += 1
                    n += 1

            def warm_fill():
                # dependency-free matmuls: fill PE-idle slots in fill-poor
                # zones so the HAM clock gate never sees a sparse window
                wf = ps_aux.tile([128, CH], F32, tag="aux", name="wf")
                for _ in range(5):
                    nc.tensor.matmul(wf[:, 0:128], wsc[:, 0:128],
                                     wsc[:, 0:128], start=True, stop=True)

            # prologue: batch 0's group-0 qkv runs alone before attention;
            # it is DMA-paced, so keep HAM warm with spin matmuls between
            for th in qkv_groups[0][0]:
                th()
                warm_fill()
            # Each batch window carries two paced fill zones:
            #   zone A (chunks 0-1): this batch's OWN group-1 qkv (it must
            #     finish before chunk 2 reads q/k/v cols >= TG)
            #   zone B (chunks 2-3): the NEXT batch's group-0 qkv
            # This shifts qkv half a batch later than the naive schedule so
            # batch B-1's ACT-bound attention still has PE fill work.
            for b in range(B):
                attn = list(attn_streams[b])
                zone_split = chunk_marks[b][2]
                if b > 0:
                    # splice the previous batch's last-chunk normalize in
                    # after a few steps (dcol DMA has landed by then)
                    attn.insert(8, fin_lasts[b - 1])
                    if zone_split >= 8:
                        zone_split += 1
                fill_a = list(qkv_groups[b][1])
                fill_b = list(qkv_groups[b + 1][0]) if b + 1 < B else \
                    [warm_fill] * 14
                zones = [(0, zone_split, fill_a),
                         (zone_split, len(attn), fill_b)]
                for z0, z1, fill in zones:
                    sub = attn[z0:z1]
                    wsum = sum(getattr(th, "weight", 1.0) for th in sub)
                    rate = len(fill) / wsum if wsum else 0.0
                    credit = 0.0
                    qpos = 0
                    for th in sub:
                        th()
                        drain_clock[0] += 1
                        credit += rate * getattr(th, "weight", 1.0)
                        while credit >= 1.0 and qpos < len(fill):
                            fill[qpos]()
                            qpos += 1
                            credit -= 1.0
                        drain_proj(limit=3, reserve=RESERVE)
                    while qpos < len(fill):
                        fill[qpos]()
                        qpos += 1
            # ---- tail: drain part of the reserve (dense, dependency-free
            # PE work) while the last chunk's dcol DMA lands, then run its
            # normalize and flush everything.  Attention is over, so its
            # PSUM banks are free: alternate po between two pools and the
            # copies between both copy engines for a deeper pipeline.
            tk = [0]

            def tail_drain(limit):
                n0 = proj_cursor[0]
                while proj_cursor[0] < len(proj_pending) \
                        and proj_cursor[0] - n0 < limit:
                    at, b, m, ch = proj_pending[proj_cursor[0]]
                    proj_emit_half(b, m, ch,
                                   pool=(ps_s if tk[0] % 2 else ps_aux),
                                   sc=(tk[0] % 4 >= 2), dma_half=True)
                    proj_cursor[0] += 1
                    tk[0] += 1
                    if tk[0] % 4 == 0:
                        warm_fill()
            tail_drain(8)
            fin_lasts[B - 1]()
            tail_drain(10 ** 9)

    nc.compile()
    return nc


def host_inputs(cfg: Cfg, x, W_attn, W_proj, h0, hpc=2):
    """Per-core input dict for the core owning heads [h0, h0+hpc)."""
    C, D = cfg.C, cfg.D
    assert hpc * D == 128
    bf = ml_dtypes.bfloat16

    def wpack(Wcols):  # [C, 128] -> [128, n_ct*128] (c-tile minor)
        return np.ascontiguousarray(
            Wcols.reshape(cfg.n_ct, 128, 128).transpose(1, 0, 2)
            .reshape(128, cfg.n_ct * 128)).astype(bf)

    cols = np.concatenate([np.arange(h * D, (h + 1) * D)
                           for h in range(h0, h0 + hpc)])
    trm = np.triu(np.ones((128, 128)), k=1) * -60000.0
    return {
        "wq": wpack(W_attn[:, cols]),
        "wk": wpack(W_attn[:, C + cols]),
        "wv": wpack(W_attn[:, 2 * C + cols]),
        "wp": np.ascontiguousarray(W_proj[cols, :]).astype(bf),
        "trm": trm.astype(bf),
        "idn": np.concatenate([np.eye(128), np.eye(128)], axis=1).astype(bf),
        "idf": np.eye(128, dtype=np.float32),
    }


_NC_CACHE = {}


def kernel(x, W_attn, W_proj):
    x = np.asarray(x)
    W_attn = np.asarray(W_attn)
    W_proj = np.asarray(W_proj)
    B, T, C = x.shape
    cfg = Cfg(B=B, T=T, C=C)
    key = (B, T, C)
    if key not in _NC_CACHE:
        _NC_CACHE[key] = build(cfg)
    nc = _NC_CACHE[key]

    xT = np.ascontiguousarray(x.reshape(cfg.BT, C).T).astype(ml_dtypes.bfloat16)
    in_maps = []
    for core in range(N_CORES):
        m = host_inputs(cfg, x, W_attn, W_proj, h0=2 * core)
        m["xT"] = xT
        in_maps.append(m)

    res = run_bass_kernel_spmd(nc, in_maps, core_ids=list(range(N_CORES)))
    out = np.zeros((cfg.BT, C), dtype=np.float64)
    for core in range(N_CORES):
        out += res.results[core]["outp"].astype(np.float64)
    return out.astype(np.float32).reshape(B, T, C)
